# revision 57
# baseline (speedup 1.0000x reference)
"""DAGMM forward kernel for 8 Trainium2 NeuronCores (Bass/Tile).

Strategy (data-parallel over batch N, per the sharding hint):
  Phase 1 (device): per-core shard of x runs the encoder + estimation nets
    entirely in "transposed" layout (features on partitions, batch rows on
    the moving free dim, fp32r matmuls).  Per 128-row block the kernel
    transposes z / softmax-logits back to natural layout on the PE and
    accumulates the GMM sufficient statistics
        stats = U^T @ [z*rZ | (z (x) z)*rZ | rZ | rZ]   (U = exp(logits))
    into a single PSUM bank across the whole shard (softmax normalisation
    1/Z is folded into the moving operand, so gamma is never materialised).
    Outputs: zT [8, NS] (z transposed, fp32r-rounded) and stats [16, 74].
  Host glue (tiny, fp64): all-reduce stats over the 8 cores, form
    phi/mu/sigma, cholesky, sigma^-1, logdet; fold everything into a
    [72, 16] stationary matrix + [16] bias for the energy.
  Phase 2 (device): per-core, logits = Wst^T @ [zT ; z (x) z] + c in one
    matmul (the outer products are built on-device from zT with two 0/1
    stationary matmuls + one DVE multiply), then exp / column-sum / ln.
    prob = -ln(sum) with the negation done on host during unsharding.
"""

import sys

for _p in ("/opt/trn_rl_repo",):
    if _p not in sys.path:
        sys.path.insert(0, _p)

import numpy as np

import concourse.bass as bass
import concourse.mybir as mybir
import concourse.tile as tile
from concourse import bass_utils
from concourse.vector_clock import ScopedClock
from contextlib import ExitStack

F32 = mybir.dt.float32
F32R = mybir.dt.float32r
AF = mybir.ActivationFunctionType
MUL = mybir.AluOpType.mult

N_TOTAL = 262144
N_CORES = 8
NS = N_TOTAL // N_CORES          # 32768 rows per core
TILE_ROWS = 512
NT = NS // TILE_ROWS             # 64 tiles per core
K = 16                           # mixture components
DZ = 8                           # latent dim
EPS = 1e-6


class _TileContextP(tile.TileContext):
    """TileContext whose tail drain splits sem waits one-per-instruction
    (this walrus build rejects >1 sync wait per instruction)."""

    def _drain_and_barrier(self, tick_clock, wait_clock):
        nc = self.nc
        vclock = tick_clock.global_clock
        for proc in range(len(vclock)):
            t = vclock[proc]
            if t > 0:
                sub = ScopedClock()
                sub.require_at_least(None, proc, t)
                nop = nc.sync.nop()
                wait_clock.add_sem_waits(nop.ins, sub)
        nc.sync.drain()
        nc.all_engine_barrier()
        assert self.sems is not None
        popped = nc._tile_sem_poison_stack.pop()
        assert popped is self._sem_poison
        nc.clear_and_free_semaphores(list(self.sems.allocated().values()))
        nc.all_engine_barrier()


def _split_waits(nc, max_waits=1):
    """Move extra sem waits onto same-engine NOPs inserted just before the
    offending instruction (walrus here allows a single sync wait per inst)."""
    n_new = 0
    for f in nc.m.functions:
        for bb in f.blocks:
            insts = bb.instructions
            i = 0
            while i < len(insts):
                inst = insts[i]
                si = getattr(inst, "sync_info", None)
                if si is not None and si.on_wait and len(si.on_wait) > max_waits:
                    waits = list(si.on_wait)
                    keep = waits[-max_waits:]
                    extra = waits[:-max_waits]
                    nops = []
                    for w in extra:
                        n_new += 1
                        nop = mybir.InstNoOp(name=f"I-waitsplit-{n_new}", ins=[], outs=[])
                        nop.engine = inst.engine
                        nop.sync_info = mybir.SyncInfo(on_wait=[w], on_update=[])
                        nops.append(nop)
                    inst.sync_info = mybir.SyncInfo(
                        on_wait=keep, on_update=list(si.on_update or []))
                    for j, nop in enumerate(nops):
                        insts.insert(i + j, nop)
                    i += len(nops)
                i += 1
    return n_new


def build_phase1(nt=NT):
    ns = nt * TILE_ROWS
    nc = bass.Bass()
    x = nc.dram_tensor("x", [ns, 128], F32, kind="ExternalInput")
    W0 = nc.dram_tensor("W0", [128, 512], F32R, kind="ExternalInput")
    W1 = nc.dram_tensor("W1", [512, 256], F32R, kind="ExternalInput")
    W2 = nc.dram_tensor("W2", [256, 64], F32R, kind="ExternalInput")
    W3 = nc.dram_tensor("W3", [64, 8], F32R, kind="ExternalInput")
    WE0 = nc.dram_tensor("WE0", [64, 128], F32R, kind="ExternalInput")
    E1 = nc.dram_tensor("E1", [128, 64], F32R, kind="ExternalInput")
    E2 = nc.dram_tensor("E2", [64, 16], F32R, kind="ExternalInput")
    b0 = nc.dram_tensor("b0", [512], F32, kind="ExternalInput")
    b1 = nc.dram_tensor("b1", [256], F32, kind="ExternalInput")
    b2 = nc.dram_tensor("b2", [64], F32, kind="ExternalInput")
    b3 = nc.dram_tensor("b3", [8], F32, kind="ExternalInput")
    bE0 = nc.dram_tensor("bE0", [128], F32, kind="ExternalInput")
    eb1 = nc.dram_tensor("eb1", [64], F32, kind="ExternalInput")
    eb2 = nc.dram_tensor("eb2", [16], F32, kind="ExternalInput")
    idf = nc.dram_tensor("idf", [128, 128], F32, kind="ExternalInput")
    idr = nc.dram_tensor("idr", [16, 16], F32R, kind="ExternalInput")
    ones2c = nc.dram_tensor("ones2c", [128, 2], F32R, kind="ExternalInput")
    zT = nc.dram_tensor("zT", [8, ns], F32R, kind="ExternalOutput")
    stats = nc.dram_tensor("stats", [16, 72], F32, kind="ExternalOutput")
    gsum = nc.dram_tensor("gsum", [64, 2], F32, kind="ExternalOutput")

    with _TileContextP(nc) as tc, ExitStack() as ctx:
        const = ctx.enter_context(tc.tile_pool(name="const", bufs=1))
        acts = ctx.enter_context(tc.tile_pool(name="acts", bufs=3))
        zg = ctx.enter_context(tc.tile_pool(name="zg", bufs=18))
        sm = ctx.enter_context(tc.tile_pool(name="sm", bufs=6))
        pmm = ctx.enter_context(tc.tile_pool(name="pmm", bufs=5, space="PSUM"))
        pnat = ctx.enter_context(tc.tile_pool(name="pnat", bufs=2, space="PSUM"))
        pstat = ctx.enter_context(tc.tile_pool(name="pstat", bufs=1, space="PSUM"))

        W0s = const.tile([128, 512], F32R)
        nc.sync.dma_start(W0s[:], W0[:])
        W1s = const.tile([128, 1024], F32R)
        for k in range(4):
            nc.sync.dma_start(W1s[:, 256 * k:256 * (k + 1)], W1[128 * k:128 * (k + 1), :])
        W2s = const.tile([128, 128], F32R)
        for k in range(2):
            nc.sync.dma_start(W2s[:, 64 * k:64 * (k + 1)], W2[128 * k:128 * (k + 1), :])
        W3s = const.tile([64, 8], F32R)
        nc.sync.dma_start(W3s[:], W3[:])
        WE0s = const.tile([64, 128], F32R)
        nc.sync.dma_start(WE0s[:], WE0[:])
        E1s = const.tile([128, 64], F32R)
        nc.sync.dma_start(E1s[:], E1[:])
        idfs = const.tile([128, 128], F32)
        nc.sync.dma_start(idfs[:], idf[:])
        idrs = const.tile([16, 16], F32R)
        nc.sync.dma_start(idrs[:], idr[:])
        b0s = const.tile([128, 4], F32)
        nc.sync.dma_start(b0s[:], b0[:].rearrange("(c p) -> p c", p=128))
        b1s = const.tile([128, 2], F32)
        nc.sync.dma_start(b1s[:], b1[:].rearrange("(c p) -> p c", p=128))
        b2s = const.tile([64, 1], F32)
        nc.sync.dma_start(b2s[:], b2[:].rearrange("(p c) -> p c", c=1))
        b3s = const.tile([8, 1], F32)
        nc.sync.dma_start(b3s[:], b3[:].rearrange("(p c) -> p c", c=1))
        bE0s = const.tile([128, 1], F32)
        nc.sync.dma_start(bE0s[:], bE0[:].rearrange("(p c) -> p c", c=1))
        eb1s = const.tile([64, 1], F32)
        nc.sync.dma_start(eb1s[:], eb1[:].rearrange("(p c) -> p c", c=1))
        E2sb = const.tile([64, 16], F32R)
        nc.sync.dma_start(E2sb[:], E2[:])
        eb2s = const.tile([16, 1], F32)
        nc.sync.dma_start(eb2s[:], eb2[:].rearrange("(p c) -> p c", c=1))
        on2c = const.tile([128, 2], F32R)
        nc.sync.dma_start(on2c[:], ones2c[:])

        statbank = pstat.tile([64, 74], F32)
        statp = statbank[0:16, 0:72]
        gsp = statbank[0:64, 72:74]

        # Process tiles in groups of GB: first all tanh-stage work for the
        # group (ACT stays on the tanh table set), then all exp/stats work
        # (one switch to the exp set per group) — the ACT table reload costs
        # ~2.7us, so per-tile alternation would dominate the kernel.
        GB = 16
        n_groups = nt // GB
        assert nt % GB == 0
        def _head(t):
            # DMA + x transposes + PSUM->SBUF copy
            r0 = t * TILE_ROWS
            xn = acts.tile([128, 512], F32, tag="xn")
            nc.sync.dma_start(
                xn[:].rearrange("p (c f) -> p c f", c=4),
                x[r0:r0 + 512, :].rearrange("(c p) f -> p c f", p=128))
            xtp = pmm.tile([128, 512], F32, tag="mm")
            for c in range(4):
                nc.tensor.transpose(xtp[:, 128 * c:128 * (c + 1)],
                                    xn[:, 128 * c:128 * (c + 1)], idfs[:])
            xts = acts.tile([128, 512], F32R, tag="xts")
            nc.vector.tensor_copy(xts[:], xtp[:])
            return xts

        def _mid(xts):
            # L0 + L1 (+ tanh)
            h0s = acts.tile([128, 2048], F32R, tag="h0")
            for m in range(4):
                hp = pmm.tile([128, 512], F32, tag="mm")
                nc.tensor.matmul(hp[:], W0s[:, 128 * m:128 * (m + 1)], xts[:],
                                 start=True, stop=True)
                nc.scalar.activation(h0s[:, 512 * m:512 * (m + 1)], hp[:], AF.Tanh,
                                     bias=b0s[:, m:m + 1])
            h1s = acts.tile([128, 1024], F32R, tag="h1")
            for m in range(2):
                hp = pmm.tile([128, 512], F32, tag="mm")
                for k in range(4):
                    nc.tensor.matmul(hp[:], W1s[:, 256 * k + 128 * m: 256 * k + 128 * m + 128],
                                     h0s[:, 512 * k:512 * (k + 1)],
                                     start=(k == 0), stop=(k == 3))
                nc.scalar.activation(h1s[:, 512 * m:512 * (m + 1)], hp[:], AF.Tanh,
                                     bias=b1s[:, m:m + 1])
            return h1s

        def _back(t, h1s):
            # L2 .. estimation net tail
            r0 = t * TILE_ROWS
            h2p = pmm.tile([64, 512], F32, tag="mm")
            for k in range(2):
                nc.tensor.matmul(h2p[:], W2s[:, 64 * k:64 * (k + 1)],
                                 h1s[:, 512 * k:512 * (k + 1)],
                                 start=(k == 0), stop=(k == 1))
            h2s = acts.tile([64, 512], F32R, tag="h2")
            nc.scalar.activation(h2s[:], h2p[:], AF.Tanh, bias=b2s[:, 0:1])

            zp = pmm.tile([8, 512], F32, tag="mm")
            nc.tensor.matmul(zp[:], W3s[:], h2s[:], start=True, stop=True)
            zTs = zg.tile([8, 512], F32R, tag="zTs")
            nc.vector.tensor_scalar_add(zTs[:], zp[:], b3s[:, 0:1])
            nc.scalar.dma_start(zT[:, r0:r0 + 512], zTs[:])

            g0p = pmm.tile([128, 512], F32, tag="mm")
            nc.tensor.matmul(g0p[:], WE0s[:], h2s[:], start=True, stop=True)
            g0s = acts.tile([128, 512], F32R, tag="g0")
            nc.scalar.activation(g0s[:], g0p[:], AF.Tanh, bias=bE0s[:, 0:1])
            g1p = pmm.tile([64, 512], F32, tag="mm")
            nc.tensor.matmul(g1p[:], E1s[:], g0s[:], start=True, stop=True)
            g1s = acts.tile([64, 512], F32R, tag="g1")
            nc.scalar.activation(g1s[:], g1p[:], AF.Tanh, bias=eb1s[:, 0:1])
            gep = pmm.tile([16, 512], F32, tag="mm")
            nc.tensor.matmul(gep[:], E2sb[:], g1s[:], start=True, stop=True)
            ges = zg.tile([16, 512], F32R, tag="ges")
            nc.scalar.activation(ges[:], gep[:], AF.Identity, bias=eb2s[:, 0:1])
            return zTs, ges

        for g in range(n_groups):
            zlist, glist = [], []
            # Software-pipelined emission: per-engine streams interleave tile
            # t's front with tile t-1's back so the in-order engines always
            # have off-critical-path work between ladder steps.
            pend = None
            for tt in range(GB):
                t = g * GB + tt
                xts = _head(t)
                if pend is not None:
                    zTs, ges = _back(*pend)
                    zlist.append(zTs)
                    glist.append(ges)
                h1s = _mid(xts)
                pend = (t, h1s)
            zTs, ges = _back(*pend)
            zlist.append(zTs)
            glist.append(ges)

            def _nat(tt):
                # All 4 blocks' z/glog transposed into one PSUM tile.
                zTs = zlist[tt]
                ges = glist[tt]
                natp = pnat.tile([128, 96], F32R, tag="nat")
                for c in range(4):
                    nc.tensor.transpose(natp[:, 24 * c:24 * c + 8],
                                        zTs[:, 128 * c:128 * (c + 1)],
                                        idrs[0:8, 0:8])
                    nc.tensor.transpose(natp[:, 24 * c + 8:24 * c + 24],
                                        ges[:, 128 * c:128 * (c + 1)],
                                        idrs[:])
                return natp

            def _stats(tt, natp):
                t = g * GB + tt
                # A single Exp covers all 4 blocks' logits; Z via one reduce.
                U4 = sm.tile([128, 64], F32, tag="U4")
                nc.scalar.activation(
                    U4[:].rearrange("p (c w) -> p c w", w=16),
                    natp[:].bitcast(F32).rearrange("p (c w) -> p c w", w=24)[:, :, 8:24],
                    AF.Exp)
                Z4 = sm.tile([128, 4], F32, tag="Z4")
                nc.vector.tensor_reduce(
                    Z4[:], U4[:].rearrange("p (c w) -> p c w", w=16),
                    mybir.AxisListType.X, mybir.AluOpType.add)
                rZ4 = sm.tile([128, 4], F32, tag="rZ4")
                nc.vector.reciprocal(rZ4[:], Z4[:])
                G4 = sm.tile([128, 64], F32R, tag="G4")
                nc.vector.tensor_tensor(
                    G4[:].rearrange("p (c w) -> p c w", w=16),
                    U4[:].rearrange("p (c w) -> p c w", w=16),
                    rZ4[:].unsqueeze(2).broadcast_to([128, 4, 16]),
                    MUL)
                first = (t == 0)
                last = (t == nt - 1)
                nc.tensor.matmul(gsp, G4[:], on2c[:], start=first, stop=last,
                                 skip_group_check=True)
                for c in range(4):
                    zsl = natp[:, 24 * c:24 * c + 8].bitcast(F32)
                    Ft = sm.tile([128, 72], F32R, tag="Ft")
                    nc.vector.tensor_copy(Ft[:, 0:8], zsl)
                    nc.vector.tensor_tensor(
                        Ft[:, 8:72].rearrange("p (l m) -> p l m", l=8),
                        Ft[:, 0:8].bitcast(F32).unsqueeze(1).broadcast_to([128, 8, 8]),
                        zsl.unsqueeze(2).broadcast_to([128, 8, 8]),
                        MUL)
                    firstc = (t == 0 and c == 0)
                    lastc = (t == nt - 1 and c == 3)
                    nc.tensor.matmul(statp, G4[:, 16 * c:16 * (c + 1)], Ft[:],
                                     start=firstc, stop=lastc, skip_group_check=True)

            pend2 = None
            for tt in range(GB):
                natp = _nat(tt)
                if pend2 is not None:
                    _stats(*pend2)
                pend2 = (tt, natp)
            _stats(*pend2)

        stats_sb = const.tile([16, 72], F32)
        nc.vector.tensor_copy(stats_sb[:], statp)
        nc.sync.dma_start(stats[:], stats_sb[:])
        gs_sb = const.tile([64, 2], F32)
        nc.vector.tensor_copy(gs_sb[:], gsp)
        nc.sync.dma_start(gsum[:], gs_sb[:])

    _split_waits(nc)
    return nc


def build_phase2(nt=NT):
    ns = nt * TILE_ROWS
    nc = bass.Bass()
    zT = nc.dram_tensor("zT", [8, ns], F32R, kind="ExternalInput")
    Ast = nc.dram_tensor("Ast", [8, 128], F32R, kind="ExternalInput")
    vb = nc.dram_tensor("vb", [128], F32, kind="ExternalInput")
    Gm = nc.dram_tensor("Gm", [128, 16], F32R, kind="ExternalInput")
    cvec = nc.dram_tensor("cvec", [16], F32, kind="ExternalInput")
    ones2 = nc.dram_tensor("ones2", [16, 2], F32R, kind="ExternalInput")
    lnq = nc.dram_tensor("lnq", [1, ns], F32, kind="ExternalOutput")

    with _TileContextP(nc) as tc, ExitStack() as ctx:
        const = ctx.enter_context(tc.tile_pool(name="const", bufs=1))
        sb = ctx.enter_context(tc.tile_pool(name="sb", bufs=3))
        pV = ctx.enter_context(tc.tile_pool(name="pV", bufs=2, space="PSUM"))
        pq = ctx.enter_context(tc.tile_pool(name="pq", bufs=2, space="PSUM"))
        ps = ctx.enter_context(tc.tile_pool(name="ps", bufs=2, space="PSUM"))

        Asts = const.tile([8, 128], F32R)
        nc.sync.dma_start(Asts[:], Ast[:])
        vbs = const.tile([128, 1], F32)
        nc.sync.dma_start(vbs[:], vb[:].rearrange("(p c) -> p c", c=1))
        Gs = const.tile([128, 16], F32R)
        nc.sync.dma_start(Gs[:], Gm[:])
        cs = const.tile([16, 1], F32)
        nc.sync.dma_start(cs[:], cvec[:].rearrange("(p c) -> p c", c=1))
        on2 = const.tile([16, 2], F32R)
        nc.sync.dma_start(on2[:], ones2[:])

        def _pa(t):
            r0 = t * TILE_ROWS
            zm = sb.tile([8, 512], F32R, tag="zm")
            nc.sync.dma_start(zm[:], zT[:, r0:r0 + 512])
            Vp = pV.tile([128, 512], F32, tag="Vp")
            nc.tensor.matmul(Vp[:], Asts[:], zm[:], start=True, stop=True)
            return Vp

        def _pb(t, Vp):
            r0 = t * TILE_ROWS
            Vb = sb.tile([128, 512], F32, tag="Vb")
            nc.vector.tensor_scalar_add(Vb[:], Vp[:], vbs[:, 0:1])
            V2 = sb.tile([128, 512], F32R, tag="V2")
            nc.vector.tensor_tensor(V2[:], Vb[:], Vb[:], MUL)
            qp = pq.tile([16, 512], F32, tag="qp")
            nc.tensor.matmul(qp[:], Gs[:], V2[:], start=True, stop=True)
            Ee = sb.tile([16, 512], F32R, tag="Ee")
            nc.scalar.activation(Ee[:], qp[:], AF.Exp, bias=cs[:, 0:1], scale=-0.5)
            sp = ps.tile([2, 512], F32, tag="sp")
            nc.tensor.matmul(sp[:], on2[:], Ee[:], start=True, stop=True)
            lsb = sb.tile([1, 512], F32, tag="lsb")
            nc.scalar.activation(lsb[:], sp[0:1, :], AF.Ln)
            nc.scalar.dma_start(lnq[0:1, r0:r0 + 512], lsb[0:1, :])

        pend = None
        for t in range(nt):
            Vp = _pa(t)
            if pend is not None:
                _pb(*pend)
            pend = (t, Vp)
        _pb(*pend)

    _split_waits(nc)
    return nc


_CACHE = {}


def _get_phase1():
    if "p1" not in _CACHE:
        _CACHE["p1"] = build_phase1()
    return _CACHE["p1"]


def _get_phase2():
    if "p2" not in _CACHE:
        _CACHE["p2"] = build_phase2()
    return _CACHE["p2"]


def _phase1_in_maps(inputs):
    f32 = lambda a: np.ascontiguousarray(np.asarray(a), dtype=np.float32)
    x = f32(inputs["x"])
    W3 = np.asarray(inputs["W3"], np.float64)
    E0 = np.asarray(inputs["E0"], np.float64)
    WE0 = (W3 @ E0).astype(np.float32)
    bE0 = (np.asarray(inputs["b3"], np.float64) @ E0
           + np.asarray(inputs["eb0"], np.float64)).astype(np.float32)
    shared = dict(
        W0=f32(inputs["W0"]), W1=f32(inputs["W1"]), W2=f32(inputs["W2"]),
        W3=f32(inputs["W3"]), WE0=WE0, E1=f32(inputs["E1"]), E2=f32(inputs["E2"]),
        b0=f32(inputs["b0"]), b1=f32(inputs["b1"]), b2=f32(inputs["b2"]),
        b3=f32(inputs["b3"]), bE0=bE0, eb1=f32(inputs["eb1"]), eb2=f32(inputs["eb2"]),
        idf=np.eye(128, dtype=np.float32),
        idr=np.eye(16, dtype=np.float32),
        ones2c=np.ones((128, 2), np.float32),
    )
    return [dict(shared, x=x[c * NS:(c + 1) * NS]) for c in range(N_CORES)]


def _host_glue(stats_list, gsum_list):
    """fp64 mid-computation: stats -> (Ast [8,128], vb [128], cvec [16])."""
    stats = np.zeros((K, 72), np.float64)
    for s in stats_list:
        stats += np.asarray(s, np.float64)
    gs = np.zeros(K, np.float64)
    for g in gsum_list:
        gs += np.asarray(g, np.float64)[:, 0].reshape(4, K).sum(axis=0)
    m = stats[:, 0:8]
    S = stats[:, 8:72].reshape(K, DZ, DZ)
    phi = gs / N_TOTAL
    mu = m / gs[:, None]
    sigma = S / gs[:, None, None] - mu[:, None, :] * mu[:, :, None]
    sigma = sigma + EPS * np.eye(DZ)[None]
    L = np.linalg.cholesky(sigma)
    logdet = 2.0 * np.log(np.diagonal(L, axis1=1, axis2=2)).sum(1)
    Linv = np.stack([np.linalg.inv(L[k]) for k in range(K)])
    Ast = np.zeros((8, 128), np.float64)
    vb = np.zeros(128, np.float64)
    for k in range(K):
        Lm = Linv[k] @ mu[k]
        for j in range(DZ):
            Ast[:, k * DZ + j] = Linv[k, j, :]
            vb[k * DZ + j] = -Lm[j]
    cvec = np.log(phi) - 0.5 * (DZ * np.log(2.0 * np.pi) + logdet)
    return Ast.astype(np.float32), vb.astype(np.float32), cvec.astype(np.float32)


def _phase2_in_maps(zT_list, Ast, vb, cvec):
    Gmask = np.zeros((128, 16), np.float32)
    for p in range(128):
        Gmask[p, p // DZ] = 1.0
    shared = dict(Ast=Ast, vb=vb, Gm=Gmask, cvec=cvec,
                  ones2=np.ones((16, 2), np.float32))
    return [dict(shared, zT=np.ascontiguousarray(zT_list[c], np.float32))
            for c in range(N_CORES)]


def kernel(**inputs):
    nc1 = _get_phase1()
    in1 = _phase1_in_maps(inputs)
    r1 = bass_utils.run_bass_kernel_spmd(nc1, in1, core_ids=list(range(N_CORES)))
    zT_list = [r1.results[c]["zT"] for c in range(N_CORES)]
    stats_list = [r1.results[c]["stats"] for c in range(N_CORES)]
    gsum_list = [r1.results[c]["gsum"] for c in range(N_CORES)]

    Ast, vb, cvec = _host_glue(stats_list, gsum_list)

    nc2 = _get_phase2()
    in2 = _phase2_in_maps(zT_list, Ast, vb, cvec)
    r2 = bass_utils.run_bass_kernel_spmd(nc2, in2, core_ids=list(range(N_CORES)))

    prob = -np.concatenate([r2.results[c]["lnq"].reshape(NS) for c in range(N_CORES)])
    z = np.concatenate([zT_list[c].T for c in range(N_CORES)], axis=0)
    return prob.astype(np.float32), np.ascontiguousarray(z, np.float32)


# revision 60
# speedup vs baseline: 1.3872x; 1.3872x over previous
"""DAGMM forward kernel for 8 Trainium2 NeuronCores (Bass/Tile).

Strategy (data-parallel over batch N, per the sharding hint):
  Phase 1 (device): per-core shard of x runs the encoder + estimation nets
    entirely in "transposed" layout (features on partitions, batch rows on
    the moving free dim, fp32r matmuls).  Per 128-row block the kernel
    transposes z / softmax-logits back to natural layout on the PE and
    accumulates the GMM sufficient statistics
        stats = U^T @ [z*rZ | (z (x) z)*rZ | rZ | rZ]   (U = exp(logits))
    into a single PSUM bank across the whole shard (softmax normalisation
    1/Z is folded into the moving operand, so gamma is never materialised).
    Outputs: zT [8, NS] (z transposed, fp32r-rounded) and stats [16, 74].
  Host glue (tiny, fp64): all-reduce stats over the 8 cores, form
    phi/mu/sigma, cholesky, sigma^-1, logdet; fold everything into a
    [72, 16] stationary matrix + [16] bias for the energy.
  Phase 2 (device): per-core, logits = Wst^T @ [zT ; z (x) z] + c in one
    matmul (the outer products are built on-device from zT with two 0/1
    stationary matmuls + one DVE multiply), then exp / column-sum / ln.
    prob = -ln(sum) with the negation done on host during unsharding.
"""

import sys

for _p in ("/opt/trn_rl_repo",):
    if _p not in sys.path:
        sys.path.insert(0, _p)

import numpy as np

import concourse.bass as bass
import concourse.mybir as mybir
import concourse.tile as tile
from concourse import bass_utils
from concourse.vector_clock import ScopedClock
from contextlib import ExitStack

F32 = mybir.dt.float32
F32R = mybir.dt.float32r
AF = mybir.ActivationFunctionType
MUL = mybir.AluOpType.mult

N_TOTAL = 262144
N_CORES = 8
NS = N_TOTAL // N_CORES          # 32768 rows per core
TILE_ROWS = 512
NT = NS // TILE_ROWS             # 64 tiles per core
K = 16                           # mixture components
DZ = 8                           # latent dim
EPS = 1e-6


class _TileContextP(tile.TileContext):
    """TileContext whose tail drain splits sem waits one-per-instruction
    (this walrus build rejects >1 sync wait per instruction)."""

    def _drain_and_barrier(self, tick_clock, wait_clock):
        nc = self.nc
        vclock = tick_clock.global_clock
        for proc in range(len(vclock)):
            t = vclock[proc]
            if t > 0:
                sub = ScopedClock()
                sub.require_at_least(None, proc, t)
                nop = nc.sync.nop()
                wait_clock.add_sem_waits(nop.ins, sub)
        nc.sync.drain()
        nc.all_engine_barrier()
        assert self.sems is not None
        popped = nc._tile_sem_poison_stack.pop()
        assert popped is self._sem_poison
        nc.clear_and_free_semaphores(list(self.sems.allocated().values()))
        nc.all_engine_barrier()


def _split_waits(nc, max_waits=1):
    """Move extra sem waits onto same-engine NOPs inserted just before the
    offending instruction (walrus here allows a single sync wait per inst)."""
    n_new = 0
    for f in nc.m.functions:
        for bb in f.blocks:
            insts = bb.instructions
            i = 0
            while i < len(insts):
                inst = insts[i]
                si = getattr(inst, "sync_info", None)
                if si is not None and si.on_wait and len(si.on_wait) > max_waits:
                    waits = list(si.on_wait)
                    keep = waits[-max_waits:]
                    extra = waits[:-max_waits]
                    nops = []
                    for w in extra:
                        n_new += 1
                        nop = mybir.InstNoOp(name=f"I-waitsplit-{n_new}", ins=[], outs=[])
                        nop.engine = inst.engine
                        nop.sync_info = mybir.SyncInfo(on_wait=[w], on_update=[])
                        nops.append(nop)
                    inst.sync_info = mybir.SyncInfo(
                        on_wait=keep, on_update=list(si.on_update or []))
                    for j, nop in enumerate(nops):
                        insts.insert(i + j, nop)
                    i += len(nops)
                i += 1
    return n_new


def build_phase1(nt=NT):
    ns = nt * TILE_ROWS
    nc = bass.Bass()
    x = nc.dram_tensor("x", [ns, 128], F32, kind="ExternalInput")
    W0 = nc.dram_tensor("W0", [128, 512], F32R, kind="ExternalInput")
    W1 = nc.dram_tensor("W1", [512, 256], F32R, kind="ExternalInput")
    W2 = nc.dram_tensor("W2", [256, 64], F32R, kind="ExternalInput")
    W3 = nc.dram_tensor("W3", [64, 8], F32R, kind="ExternalInput")
    WE0 = nc.dram_tensor("WE0", [64, 128], F32R, kind="ExternalInput")
    E1 = nc.dram_tensor("E1", [128, 64], F32R, kind="ExternalInput")
    E2 = nc.dram_tensor("E2", [64, 16], F32R, kind="ExternalInput")
    b0 = nc.dram_tensor("b0", [512], F32, kind="ExternalInput")
    b1 = nc.dram_tensor("b1", [256], F32, kind="ExternalInput")
    b2 = nc.dram_tensor("b2", [64], F32, kind="ExternalInput")
    b3 = nc.dram_tensor("b3", [8], F32, kind="ExternalInput")
    bE0 = nc.dram_tensor("bE0", [128], F32, kind="ExternalInput")
    eb1 = nc.dram_tensor("eb1", [64], F32, kind="ExternalInput")
    eb2 = nc.dram_tensor("eb2", [16], F32, kind="ExternalInput")
    idf = nc.dram_tensor("idf", [128, 128], F32, kind="ExternalInput")
    idr = nc.dram_tensor("idr", [16, 16], F32R, kind="ExternalInput")
    ones2c = nc.dram_tensor("ones2c", [128, 2], F32R, kind="ExternalInput")
    zT = nc.dram_tensor("zT", [8, ns], F32R, kind="ExternalOutput")
    stats = nc.dram_tensor("stats", [16, 72], F32, kind="ExternalOutput")
    gsum = nc.dram_tensor("gsum", [64, 2], F32, kind="ExternalOutput")

    with _TileContextP(nc) as tc, ExitStack() as ctx:
        const = ctx.enter_context(tc.tile_pool(name="const", bufs=1))
        acts = ctx.enter_context(tc.tile_pool(name="acts", bufs=3))
        zg = ctx.enter_context(tc.tile_pool(name="zg", bufs=18))
        sm = ctx.enter_context(tc.tile_pool(name="sm", bufs=6))
        pmm = ctx.enter_context(tc.tile_pool(name="pmm", bufs=5, space="PSUM"))
        pnat = ctx.enter_context(tc.tile_pool(name="pnat", bufs=2, space="PSUM"))
        pstat = ctx.enter_context(tc.tile_pool(name="pstat", bufs=1, space="PSUM"))

        W0s = const.tile([128, 512], F32R)
        nc.sync.dma_start(W0s[:], W0[:])
        W1s = const.tile([128, 1024], F32R)
        for k in range(4):
            nc.sync.dma_start(W1s[:, 256 * k:256 * (k + 1)], W1[128 * k:128 * (k + 1), :])
        W2s = const.tile([128, 128], F32R)
        for k in range(2):
            nc.sync.dma_start(W2s[:, 64 * k:64 * (k + 1)], W2[128 * k:128 * (k + 1), :])
        W3s = const.tile([64, 8], F32R)
        nc.sync.dma_start(W3s[:], W3[:])
        WE0s = const.tile([64, 128], F32R)
        nc.sync.dma_start(WE0s[:], WE0[:])
        E1s = const.tile([128, 64], F32R)
        nc.sync.dma_start(E1s[:], E1[:])
        idfs = const.tile([128, 128], F32)
        nc.sync.dma_start(idfs[:], idf[:])
        idrs = const.tile([16, 16], F32R)
        nc.sync.dma_start(idrs[:], idr[:])
        b0s = const.tile([128, 4], F32)
        nc.sync.dma_start(b0s[:], b0[:].rearrange("(c p) -> p c", p=128))
        b1s = const.tile([128, 2], F32)
        nc.sync.dma_start(b1s[:], b1[:].rearrange("(c p) -> p c", p=128))
        b2s = const.tile([64, 1], F32)
        nc.sync.dma_start(b2s[:], b2[:].rearrange("(p c) -> p c", c=1))
        b3s = const.tile([8, 1], F32)
        nc.sync.dma_start(b3s[:], b3[:].rearrange("(p c) -> p c", c=1))
        bE0s = const.tile([128, 1], F32)
        nc.sync.dma_start(bE0s[:], bE0[:].rearrange("(p c) -> p c", c=1))
        eb1s = const.tile([64, 1], F32)
        nc.sync.dma_start(eb1s[:], eb1[:].rearrange("(p c) -> p c", c=1))
        E2sb = const.tile([64, 16], F32R)
        nc.sync.dma_start(E2sb[:], E2[:])
        eb2s = const.tile([16, 1], F32)
        nc.sync.dma_start(eb2s[:], eb2[:].rearrange("(p c) -> p c", c=1))
        on2c = const.tile([128, 2], F32R)
        nc.sync.dma_start(on2c[:], ones2c[:])

        statbank = pstat.tile([64, 74], F32)
        statp = statbank[0:16, 0:72]
        gsp = statbank[0:64, 72:74]

        # Process tiles in groups of GB: first all tanh-stage work for the
        # group (ACT stays on the tanh table set), then all exp/stats work
        # (one switch to the exp set per group) — the ACT table reload costs
        # ~2.7us, so per-tile alternation would dominate the kernel.
        GB = 16
        n_groups = nt // GB
        assert nt % GB == 0
        def _head(t):
            # DMA + x transposes + PSUM->SBUF copy
            r0 = t * TILE_ROWS
            xn = acts.tile([128, 512], F32, tag="xn")
            nc.sync.dma_start(
                xn[:].rearrange("p (c f) -> p c f", c=4),
                x[r0:r0 + 512, :].rearrange("(c p) f -> p c f", p=128))
            xtp = pmm.tile([128, 512], F32, tag="mm")
            for c in range(4):
                nc.tensor.transpose(xtp[:, 128 * c:128 * (c + 1)],
                                    xn[:, 128 * c:128 * (c + 1)], idfs[:])
            xts = acts.tile([128, 512], F32R, tag="xts")
            nc.vector.tensor_copy(xts[:], xtp[:])
            return xts

        def _mid(xts):
            # L0 + L1 (+ tanh)
            h0s = acts.tile([128, 2048], F32R, tag="h0")
            for m in range(4):
                hp = pmm.tile([128, 512], F32, tag="mm")
                nc.tensor.matmul(hp[:], W0s[:, 128 * m:128 * (m + 1)], xts[:],
                                 start=True, stop=True)
                nc.scalar.activation(h0s[:, 512 * m:512 * (m + 1)], hp[:], AF.Tanh,
                                     bias=b0s[:, m:m + 1])
            h1s = acts.tile([128, 1024], F32R, tag="h1")
            for m in range(2):
                hp = pmm.tile([128, 512], F32, tag="mm")
                for k in range(4):
                    nc.tensor.matmul(hp[:], W1s[:, 256 * k + 128 * m: 256 * k + 128 * m + 128],
                                     h0s[:, 512 * k:512 * (k + 1)],
                                     start=(k == 0), stop=(k == 3))
                nc.scalar.activation(h1s[:, 512 * m:512 * (m + 1)], hp[:], AF.Tanh,
                                     bias=b1s[:, m:m + 1])
            return h1s

        def _back(t, h1s):
            # L2 .. estimation net tail
            r0 = t * TILE_ROWS
            h2p = pmm.tile([64, 512], F32, tag="mm")
            for k in range(2):
                nc.tensor.matmul(h2p[:], W2s[:, 64 * k:64 * (k + 1)],
                                 h1s[:, 512 * k:512 * (k + 1)],
                                 start=(k == 0), stop=(k == 1))
            h2s = acts.tile([64, 512], F32R, tag="h2")
            nc.scalar.activation(h2s[:], h2p[:], AF.Tanh, bias=b2s[:, 0:1])

            zp = pmm.tile([8, 512], F32, tag="mm")
            nc.tensor.matmul(zp[:], W3s[:], h2s[:], start=True, stop=True)
            zTs = zg.tile([8, 512], F32R, tag="zTs")
            nc.vector.tensor_scalar_add(zTs[:], zp[:], b3s[:, 0:1])
            nc.scalar.dma_start(zT[:, r0:r0 + 512], zTs[:])

            g0p = pmm.tile([128, 512], F32, tag="mm")
            nc.tensor.matmul(g0p[:], WE0s[:], h2s[:], start=True, stop=True)
            g0s = acts.tile([128, 512], F32R, tag="g0")
            nc.scalar.activation(g0s[:], g0p[:], AF.Tanh, bias=bE0s[:, 0:1])
            g1p = pmm.tile([64, 512], F32, tag="mm")
            nc.tensor.matmul(g1p[:], E1s[:], g0s[:], start=True, stop=True)
            g1s = acts.tile([64, 512], F32R, tag="g1")
            nc.scalar.activation(g1s[:], g1p[:], AF.Tanh, bias=eb1s[:, 0:1])
            gep = pmm.tile([16, 512], F32, tag="mm")
            nc.tensor.matmul(gep[:], E2sb[:], g1s[:], start=True, stop=True)
            ges = zg.tile([16, 512], F32R, tag="ges")
            nc.vector.tensor_scalar_add(ges[:], gep[:], eb2s[:, 0:1])
            return zTs, ges

        for g in range(n_groups):
            zlist, glist = [], []
            # Software-pipelined emission: per-engine streams interleave tile
            # t's front with tile t-1's back so the in-order engines always
            # have off-critical-path work between ladder steps.
            pend = None
            for tt in range(GB):
                t = g * GB + tt
                xts = _head(t)
                if pend is not None:
                    zTs, ges = _back(*pend)
                    zlist.append(zTs)
                    glist.append(ges)
                h1s = _mid(xts)
                pend = (t, h1s)
            zTs, ges = _back(*pend)
            zlist.append(zTs)
            glist.append(ges)

            def _nat(tt):
                # All 4 blocks' z/glog transposed into one PSUM tile.
                zTs = zlist[tt]
                ges = glist[tt]
                natp = pnat.tile([128, 96], F32R, tag="nat")
                for c in range(4):
                    nc.tensor.transpose(natp[:, 24 * c:24 * c + 8],
                                        zTs[:, 128 * c:128 * (c + 1)],
                                        idrs[0:8, 0:8])
                    nc.tensor.transpose(natp[:, 24 * c + 8:24 * c + 24],
                                        ges[:, 128 * c:128 * (c + 1)],
                                        idrs[:])
                return natp

            def _stats(tt, natp):
                t = g * GB + tt
                # A single Exp covers all 4 blocks' logits; Z via one reduce.
                U4 = sm.tile([128, 64], F32, tag="U4")
                nc.scalar.activation(
                    U4[:].rearrange("p (c w) -> p c w", w=16),
                    natp[:].bitcast(F32).rearrange("p (c w) -> p c w", w=24)[:, :, 8:24],
                    AF.Exp)
                Z4 = sm.tile([128, 4], F32, tag="Z4")
                nc.vector.tensor_reduce(
                    Z4[:], U4[:].rearrange("p (c w) -> p c w", w=16),
                    mybir.AxisListType.X, mybir.AluOpType.add)
                rZ4 = sm.tile([128, 4], F32, tag="rZ4")
                nc.vector.reciprocal(rZ4[:], Z4[:])
                G4 = sm.tile([128, 64], F32R, tag="G4")
                nc.vector.tensor_tensor(
                    G4[:].rearrange("p (c w) -> p c w", w=16),
                    U4[:].rearrange("p (c w) -> p c w", w=16),
                    rZ4[:].unsqueeze(2).broadcast_to([128, 4, 16]),
                    MUL)
                first = (t == 0)
                last = (t == nt - 1)
                nc.tensor.matmul(gsp, G4[:], on2c[:], start=first, stop=last,
                                 skip_group_check=True)
                for c in range(4):
                    zsl = natp[:, 24 * c:24 * c + 8].bitcast(F32)
                    Ft = sm.tile([128, 72], F32R, tag="Ft")
                    nc.vector.tensor_copy(Ft[:, 0:8], zsl)
                    nc.vector.tensor_tensor(
                        Ft[:, 8:72].rearrange("p (l m) -> p l m", l=8),
                        Ft[:, 0:8].bitcast(F32).unsqueeze(1).broadcast_to([128, 8, 8]),
                        zsl.unsqueeze(2).broadcast_to([128, 8, 8]),
                        MUL)
                    firstc = (t == 0 and c == 0)
                    lastc = (t == nt - 1 and c == 3)
                    nc.tensor.matmul(statp, G4[:, 16 * c:16 * (c + 1)], Ft[:],
                                     start=firstc, stop=lastc, skip_group_check=True)

            pend2 = None
            for tt in range(GB):
                natp = _nat(tt)
                if pend2 is not None:
                    _stats(*pend2)
                pend2 = (tt, natp)
            _stats(*pend2)

        stats_sb = const.tile([16, 72], F32)
        nc.vector.tensor_copy(stats_sb[:], statp)
        nc.sync.dma_start(stats[:], stats_sb[:])
        gs_sb = const.tile([64, 2], F32)
        nc.vector.tensor_copy(gs_sb[:], gsp)
        nc.sync.dma_start(gsum[:], gs_sb[:])

    _split_waits(nc)
    return nc


def build_phase2(nt=NT):
    ns = nt * TILE_ROWS
    nc = bass.Bass()
    zT = nc.dram_tensor("zT", [8, ns], F32R, kind="ExternalInput")
    Ast = nc.dram_tensor("Ast", [8, 128], F32R, kind="ExternalInput")
    vb = nc.dram_tensor("vb", [128], F32, kind="ExternalInput")
    Gm = nc.dram_tensor("Gm", [128, 16], F32R, kind="ExternalInput")
    cvec = nc.dram_tensor("cvec", [16], F32, kind="ExternalInput")
    ones2 = nc.dram_tensor("ones2", [16, 2], F32R, kind="ExternalInput")
    lnq = nc.dram_tensor("lnq", [1, ns], F32, kind="ExternalOutput")

    with _TileContextP(nc) as tc, ExitStack() as ctx:
        const = ctx.enter_context(tc.tile_pool(name="const", bufs=1))
        sb = ctx.enter_context(tc.tile_pool(name="sb", bufs=3))
        pV = ctx.enter_context(tc.tile_pool(name="pV", bufs=2, space="PSUM"))
        pq = ctx.enter_context(tc.tile_pool(name="pq", bufs=2, space="PSUM"))
        ps = ctx.enter_context(tc.tile_pool(name="ps", bufs=2, space="PSUM"))

        Asts = const.tile([8, 128], F32R)
        nc.sync.dma_start(Asts[:], Ast[:])
        vbs = const.tile([128, 1], F32)
        nc.sync.dma_start(vbs[:], vb[:].rearrange("(p c) -> p c", c=1))
        Gs = const.tile([128, 16], F32R)
        nc.sync.dma_start(Gs[:], Gm[:])
        cs = const.tile([16, 1], F32)
        nc.sync.dma_start(cs[:], cvec[:].rearrange("(p c) -> p c", c=1))
        on2 = const.tile([16, 2], F32R)
        nc.sync.dma_start(on2[:], ones2[:])

        def _pa(t):
            r0 = t * TILE_ROWS
            zm = sb.tile([8, 512], F32R, tag="zm")
            nc.sync.dma_start(zm[:], zT[:, r0:r0 + 512])
            Vp = pV.tile([128, 512], F32, tag="Vp")
            nc.tensor.matmul(Vp[:], Asts[:], zm[:], start=True, stop=True)
            return Vp

        def _pb(t, Vp):
            r0 = t * TILE_ROWS
            Vb = sb.tile([128, 512], F32, tag="Vb")
            nc.vector.tensor_scalar_add(Vb[:], Vp[:], vbs[:, 0:1])
            V2 = sb.tile([128, 512], F32R, tag="V2")
            nc.vector.tensor_tensor(V2[:], Vb[:], Vb[:], MUL)
            qp = pq.tile([16, 512], F32, tag="qp")
            nc.tensor.matmul(qp[:], Gs[:], V2[:], start=True, stop=True)
            Ee = sb.tile([16, 512], F32R, tag="Ee")
            nc.scalar.activation(Ee[:], qp[:], AF.Exp, bias=cs[:, 0:1], scale=-0.5)
            sp = ps.tile([2, 512], F32, tag="sp")
            nc.tensor.matmul(sp[:], on2[:], Ee[:], start=True, stop=True)
            lsb = sb.tile([1, 512], F32, tag="lsb")
            nc.scalar.activation(lsb[:], sp[0:1, :], AF.Ln)
            nc.scalar.dma_start(lnq[0:1, r0:r0 + 512], lsb[0:1, :])

        pend = None
        for t in range(nt):
            Vp = _pa(t)
            if pend is not None:
                _pb(*pend)
            pend = (t, Vp)
        _pb(*pend)

    _split_waits(nc)
    return nc


_CACHE = {}


def _get_phase1():
    if "p1" not in _CACHE:
        _CACHE["p1"] = build_phase1()
    return _CACHE["p1"]


def _get_phase2():
    if "p2" not in _CACHE:
        _CACHE["p2"] = build_phase2()
    return _CACHE["p2"]


def _phase1_in_maps(inputs):
    f32 = lambda a: np.ascontiguousarray(np.asarray(a), dtype=np.float32)
    x = f32(inputs["x"])
    W3 = np.asarray(inputs["W3"], np.float64)
    E0 = np.asarray(inputs["E0"], np.float64)
    WE0 = (W3 @ E0).astype(np.float32)
    bE0 = (np.asarray(inputs["b3"], np.float64) @ E0
           + np.asarray(inputs["eb0"], np.float64)).astype(np.float32)
    shared = dict(
        W0=f32(inputs["W0"]), W1=f32(inputs["W1"]), W2=f32(inputs["W2"]),
        W3=f32(inputs["W3"]), WE0=WE0, E1=f32(inputs["E1"]), E2=f32(inputs["E2"]),
        b0=f32(inputs["b0"]), b1=f32(inputs["b1"]), b2=f32(inputs["b2"]),
        b3=f32(inputs["b3"]), bE0=bE0, eb1=f32(inputs["eb1"]), eb2=f32(inputs["eb2"]),
        idf=np.eye(128, dtype=np.float32),
        idr=np.eye(16, dtype=np.float32),
        ones2c=np.ones((128, 2), np.float32),
    )
    return [dict(shared, x=x[c * NS:(c + 1) * NS]) for c in range(N_CORES)]


def _host_glue(stats_list, gsum_list):
    """fp64 mid-computation: stats -> (Ast [8,128], vb [128], cvec [16])."""
    stats = np.zeros((K, 72), np.float64)
    for s in stats_list:
        stats += np.asarray(s, np.float64)
    gs = np.zeros(K, np.float64)
    for g in gsum_list:
        gs += np.asarray(g, np.float64)[:, 0].reshape(4, K).sum(axis=0)
    m = stats[:, 0:8]
    S = stats[:, 8:72].reshape(K, DZ, DZ)
    phi = gs / N_TOTAL
    mu = m / gs[:, None]
    sigma = S / gs[:, None, None] - mu[:, None, :] * mu[:, :, None]
    sigma = sigma + EPS * np.eye(DZ)[None]
    L = np.linalg.cholesky(sigma)
    logdet = 2.0 * np.log(np.diagonal(L, axis1=1, axis2=2)).sum(1)
    Linv = np.stack([np.linalg.inv(L[k]) for k in range(K)])
    Ast = np.zeros((8, 128), np.float64)
    vb = np.zeros(128, np.float64)
    for k in range(K):
        Lm = Linv[k] @ mu[k]
        for j in range(DZ):
            Ast[:, k * DZ + j] = Linv[k, j, :]
            vb[k * DZ + j] = -Lm[j]
    cvec = np.log(phi) - 0.5 * (DZ * np.log(2.0 * np.pi) + logdet)
    return Ast.astype(np.float32), vb.astype(np.float32), cvec.astype(np.float32)


def _phase2_in_maps(zT_list, Ast, vb, cvec):
    Gmask = np.zeros((128, 16), np.float32)
    for p in range(128):
        Gmask[p, p // DZ] = 1.0
    shared = dict(Ast=Ast, vb=vb, Gm=Gmask, cvec=cvec,
                  ones2=np.ones((16, 2), np.float32))
    return [dict(shared, zT=np.ascontiguousarray(zT_list[c], np.float32))
            for c in range(N_CORES)]


def kernel(**inputs):
    nc1 = _get_phase1()
    in1 = _phase1_in_maps(inputs)
    r1 = bass_utils.run_bass_kernel_spmd(nc1, in1, core_ids=list(range(N_CORES)))
    zT_list = [r1.results[c]["zT"] for c in range(N_CORES)]
    stats_list = [r1.results[c]["stats"] for c in range(N_CORES)]
    gsum_list = [r1.results[c]["gsum"] for c in range(N_CORES)]

    Ast, vb, cvec = _host_glue(stats_list, gsum_list)

    nc2 = _get_phase2()
    in2 = _phase2_in_maps(zT_list, Ast, vb, cvec)
    r2 = bass_utils.run_bass_kernel_spmd(nc2, in2, core_ids=list(range(N_CORES)))

    prob = -np.concatenate([r2.results[c]["lnq"].reshape(NS) for c in range(N_CORES)])
    z = np.concatenate([zT_list[c].T for c in range(N_CORES)], axis=0)
    return prob.astype(np.float32), np.ascontiguousarray(z, np.float32)
